# revision 1
# baseline (speedup 1.0000x reference)
"""Trainium2 Bass kernel for nn_LSTMHarmonizer.

Data-parallel over batch: 8 cores x 8 sequences each. Per core:
  Phase 1: gx = W_ih' @ x + b  (PE GEMM, bf16, evicted to SBUF bf16 by ACT/DVE)
  Phase 2: 1024-step LSTM scan. Per step: gx_t injected into PSUM via an
           identity matmul, 16 accumulating matmuls stream W_hh' (bf16,
           FWL), one fused sigmoid over all 4 gates (tanh folded as
           2*sigma(2x)-1 with pre-scaled g-gate rows), fused DVE
           scalar_tensor_tensor ops for the c/h updates. h is stored as
           h/2 in bf16 (compensated by 2x on W_hh and head_w).
  Phase 3: 3-head GEMM from the h history + bias add + DMA out.
"""

import contextlib
import numpy as np
import ml_dtypes

import concourse.bass as bass
import concourse.mybir as mybir
from concourse.bass_utils import run_bass_kernel_spmd

BF16 = ml_dtypes.bfloat16

B, T, D, H, V, NV = 64, 1024, 128, 256, 128, 3
G4 = 4 * H            # 1024
NC = 8                # cores
BC = B // NC          # 8 sequences per core
NVV = NV * V          # 384

_cache = {}


def build_nc(Tc=T, debug=False):
    TOK = BC * Tc
    NT3 = TOK // 128          # phase-3 token tiles
    NCH = TOK // 512          # phase-1 token chunks
    f32 = mybir.dt.float32
    bf16 = mybir.dt.bfloat16

    nc = bass.Bass()
    if debug:
        gxd_d = nc.declare_dram_parameter("gx_dbg", [128, 8 * TOK], bf16,
                                          isOutput=True)
        hhd_d = nc.declare_dram_parameter("hh_dbg", [128, 2 * TOK], bf16,
                                          isOutput=True)
        ps0_d = nc.declare_dram_parameter("ps0_dbg", [128, 64], f32,
                                          isOutput=True)
        sal_d = nc.declare_dram_parameter("sall_dbg", [128, 64], f32,
                                          isOutput=True)
    xT_d = nc.declare_dram_parameter("xT", [128, TOK], bf16, isOutput=False)
    wih_d = nc.declare_dram_parameter("wihT", [128, G4], bf16, isOutput=False)
    whh_d = nc.declare_dram_parameter("whhT", [128, 16 * 128], bf16, isOutput=False)
    hw_d = nc.declare_dram_parameter("headwT", [128, 2 * NVV], bf16, isOutput=False)
    bias_d = nc.declare_dram_parameter("biasm", [128, 8], f32, isOutput=False)
    hb_d = nc.declare_dram_parameter("headb", [128, NVV], f32, isOutput=False)
    id_d = nc.declare_dram_parameter("ident", [128, 128], bf16, isOutput=False)
    cst_d = nc.declare_dram_parameter("consts", [128, 2], f32, isOutput=False)
    z_d = nc.declare_dram_parameter("zeros16", [128, 16], f32, isOutput=False)
    lg_d = nc.declare_dram_parameter("logits", [NV, BC, Tc, V], f32, isOutput=True)

    ctx = contextlib.ExitStack()
    with ctx:
        sb = lambda name, shape, dt: ctx.enter_context(
            nc.sbuf_tensor(name, shape, dt))
        ps = lambda name, shape: ctx.enter_context(
            nc.psum_tensor(name, shape, f32))
        sem = lambda name: ctx.enter_context(nc.semaphore(name))

        xT = sb("xT_s", [128, TOK], bf16)
        wih = sb("wih_s", [128, G4], bf16)
        whh = sb("whh_s", [128, 16 * 128], bf16)
        hw = sb("hw_s", [128, 2 * NVV], bf16)
        biasm = sb("biasm_s", [128, 8], f32)
        headb = sb("headb_s", [128, NVV], f32)
        ident = sb("ident_s", [128, 128], bf16)
        gx = sb("gx_s", [128, 8 * TOK], bf16)      # (m, t, b)
        hh = sb("hh_s", [128, 2 * TOK], bf16)      # (j, t, b), holds h/2
        sall = sb("sall_s", [128, 64], f32)        # sigmoid of all gates
        c_t = sb("c_s", [128, 16], f32)
        sc = sb("sc_s", [128, 16], f32)
        u_t = sb("u_s", [128, 16], f32)
        v_t = sb("v_s", [128, 16], f32)
        outb = sb("outb_s", [128, 4 * NVV], f32)   # phase-3 evict slots

        cst = sb("cst_s", [128, 2], f32)   # col0=0.5 col1=2.0
        dbg_ps = sb("dbg_ps", [128, 64], f32)
        dbg_sa = sb("dbg_sa", [128, 64], f32)
        ps_big = [ps(f"psb{i}", [128, 512]) for i in range(4)]  # phase 1 & 3
        ps_g = [ps(f"psg{i}", [128, 512]) for i in range(2)]    # scan gates

        dma_in = sem("dma_in")
        mm1 = sem("mm1")
        ev1a = sem("ev1a")
        ev1d = sem("ev1d")
        s_mm = sem("s_mm")
        s_act = sem("s_act")
        s_dvec = sem("s_dvec")
        s_actc = sem("s_actc")
        s_h = sem("s_h")
        s_dd = sem("s_dd")
        mm3 = sem("mm3")
        ev3 = sem("ev3")
        dma_out = sem("dma_out")

        ALU = mybir.AluOpType
        AF = mybir.ActivationFunctionType

        gx_v = gx[:].rearrange("p (t m b) -> p t m b", t=Tc, m=8, b=BC)

        def gx_evict_view(m, ch):
            # [128, (t=64, b=8)] destination for phase-1 psum tile (m, ch)
            return gx_v[:, ch * 64:(ch + 1) * 64, m, :]

        def gx_step_ap(t):
            # contiguous [128, 64] slice (m, b) at step t
            return gx[:, t * 64:(t + 1) * 64]

        def hh_ap(j, t):
            off = j * TOK + t * BC
            return hh[:, off:off + BC]

        with nc.Block() as block:

            @block.sync
            def _(sync):
                sync.dma_start(out=xT[:], in_=xT_d[:]).then_inc(dma_in, 16)
                sync.dma_start(out=wih[:], in_=wih_d[:]).then_inc(dma_in, 16)
                sync.dma_start(out=whh[:], in_=whh_d[:]).then_inc(dma_in, 16)
                sync.dma_start(out=hw[:], in_=hw_d[:]).then_inc(dma_in, 16)
                sync.dma_start(out=biasm[:], in_=bias_d[:]).then_inc(dma_in, 16)
                sync.dma_start(out=headb[:], in_=hb_d[:]).then_inc(dma_in, 16)
                sync.dma_start(out=ident[:], in_=id_d[:]).then_inc(dma_in, 16)
                sync.dma_start(out=cst[:], in_=cst_d[:]).then_inc(dma_in, 16)
                sync.dma_start(out=c_t[:], in_=z_d[:]).then_inc(dma_in, 16)
                # phase 3 output DMAs
                for tk in range(NT3):
                    sync.wait_ge(ev3, tk + 1)
                    for n in range(NV):
                        dview = lg_d[n, :, tk * 16:(tk + 1) * 16, :].rearrange(
                            "b t v -> t b v")
                        slot = outb[:, (tk % 4) * NVV + n * V:
                                    (tk % 4) * NVV + (n + 1) * V]
                        sync.dma_start(out=dview, in_=slot).then_inc(dma_out, 16)
                sync.wait_ge(dma_out, 48 * NT3)
                if debug:
                    sync.dma_start(out=gxd_d[:], in_=gx[:]).then_inc(dma_out, 16)
                    sync.dma_start(out=hhd_d[:], in_=hh[:]).then_inc(dma_out, 16)
                    sync.dma_start(out=ps0_d[:], in_=dbg_ps[:]).then_inc(dma_out, 16)
                    sync.dma_start(out=sal_d[:], in_=dbg_sa[:]).then_inc(dma_out, 16)
                    sync.wait_ge(dma_out, 48 * NT3 + 64)

            @block.tensor
            def _(tensor):
                tensor.wait_ge(dma_in, 144)
                # ---- phase 1: gx GEMM (m-outer, chunk-inner) ----
                for m in range(8):
                    for ch in range(NCH):
                        idx = m * NCH + ch
                        if idx >= 4:
                            j = idx - 4
                            if j % 2 == 0:
                                tensor.wait_ge(ev1a, j // 2 + 1)
                            else:
                                tensor.wait_ge(ev1d, (j + 1) // 2)
                        tensor.matmul(
                            ps_big[idx % 4][:, :512],
                            lhsT=wih[:, m * 128:(m + 1) * 128],
                            rhs=xT[:, ch * 512:(ch + 1) * 512],
                            start=True, stop=True,
                        ).then_inc(mm1, 1)
                # ---- phase 2: scan ----
                tensor.wait_ge(ev1a, 4 * NCH)
                tensor.wait_ge(ev1d, 4 * NCH)
                for t in range(Tc):
                    bank = ps_g[t % 2]
                    if t >= 2:
                        tensor.wait_ge(s_act, t - 1)
                    ins0 = tensor.matmul(
                        bank[:, :64], lhsT=ident[:], rhs=gx_step_ap(t),
                        start=True, stop=(t == 0), skip_group_check=True,
                    )
                    if t >= 1:
                        for k in range(2):
                            tensor.wait_ge(s_h, 2 * t - 1 + k)
                            for m in range(8):
                                ins = tensor.matmul(
                                    bank[:, m * 8:(m + 1) * 8],
                                    lhsT=whh[:, (k * 8 + m) * 128:(k * 8 + m + 1) * 128],
                                    rhs=hh_ap(k, t - 1),
                                    start=False, stop=(k == 1),
                                    skip_group_check=True,
                                )
                        ins.then_inc(s_mm, 1)
                    else:
                        ins0.then_inc(s_mm, 1)
                # ---- phase 3: heads ----
                tensor.wait_ge(s_h, 2 * Tc)
                for tk in range(NT3):
                    if tk >= 4:
                        tensor.wait_ge(ev3, tk - 3)
                    tensor.matmul(
                        ps_big[tk % 4][:, :NVV],
                        lhsT=hh[:, tk * 128:tk * 128 + 128],
                        rhs=hw[:, :NVV], start=True, stop=False,
                        skip_group_check=True,
                    )
                    tensor.matmul(
                        ps_big[tk % 4][:, :NVV],
                        lhsT=hh[:, TOK + tk * 128:TOK + tk * 128 + 128],
                        rhs=hw[:, NVV:2 * NVV], start=False, stop=True,
                        skip_group_check=True,
                    ).then_inc(mm3, 1)

            @block.scalar
            def _(scalar):
                scalar.wait_ge(dma_in, 144)
                # phase-1 evicts: even tiles
                for idx in range(0, 8 * NCH, 2):
                    m, ch = idx // NCH, idx % NCH
                    scalar.wait_ge(mm1, idx + 1)
                    scalar.activation(
                        out=gx_evict_view(m, ch),
                        in_=ps_big[idx % 4][:, :512].rearrange(
                            "p (t b) -> p t b", t=64, b=BC),
                        func=AF.Identity, bias=biasm[:, m:m + 1],
                    ).then_inc(ev1a, 1)
                # scan
                for t in range(Tc):
                    scalar.wait_ge(s_mm, t + 1)
                    scalar.activation(
                        out=sall[:], in_=ps_g[t % 2][:, :64], func=AF.Sigmoid,
                    ).then_inc(s_act, 1)
                    scalar.wait_ge(s_dvec, t + 1)
                    scalar.activation(
                        out=sc[:], in_=c_t[:], func=AF.Sigmoid, scale=2.0,
                    ).then_inc(s_actc, 1)

            @block.vector
            def _(vector):
                vector.wait_ge(dma_in, 144)
                # phase-1 evicts: odd tiles
                for idx in range(1, 8 * NCH, 2):
                    m, ch = idx // NCH, idx % NCH
                    vector.wait_ge(mm1, idx + 1)
                    vector.tensor_scalar_add(
                        gx_evict_view(m, ch),
                        ps_big[idx % 4][:, :512].rearrange(
                            "p (t b) -> p t b", t=64, b=BC),
                        biasm[:, m:m + 1],
                    ).then_inc(ev1d, 1)
                if debug:
                    vector.wait_ge(s_act, 1)
                    vector.tensor_copy(dbg_ps[:], ps_g[0][:, :64])
                    vector.tensor_copy(dbg_sa[:], sall[:])
                # scan: sall layout (m,b): m: 0,1=i  2,3=f  4,5=o  6,7=g
                for t in range(Tc):
                    vector.wait_ge(s_act, t + 1)
                    if t >= 1:
                        vector.wait_ge(s_dvec, t)
                    vector.scalar_tensor_tensor(
                        out=u_t[:], in0=sall[:, 48:64], scalar=cst[:, 0:1],
                        in1=sall[:, 0:16], op0=ALU.subtract, op1=ALU.mult)
                    vector.tensor_tensor(
                        out=v_t[:], in0=sall[:, 16:32], in1=c_t[:],
                        op=ALU.mult).then_inc(s_dd, 1)
                    vector.wait_ge(s_dd, t + 1)
                    vector.scalar_tensor_tensor(
                        out=c_t[:], in0=u_t[:], scalar=cst[:, 1:2], in1=v_t[:],
                        op0=ALU.mult, op1=ALU.add).then_inc(s_dvec, 1)
                    vector.wait_ge(s_actc, t + 1)
                    for j in range(2):
                        vector.scalar_tensor_tensor(
                            out=hh_ap(j, t), in0=sc[:, j * 8:(j + 1) * 8],
                            scalar=cst[:, 0:1],
                            in1=sall[:, 32 + j * 8:32 + (j + 1) * 8],
                            op0=ALU.subtract, op1=ALU.mult).then_inc(s_h, 1)
                # phase-3 evicts
                for tk in range(NT3):
                    vector.wait_ge(mm3, tk + 1)
                    if tk >= 4:
                        vector.wait_ge(dma_out, 48 * (tk - 3))
                    slot = outb[:, (tk % 4) * NVV:(tk % 4 + 1) * NVV]
                    vector.tensor_tensor(
                        out=slot, in0=ps_big[tk % 4][:, :NVV], in1=headb[:],
                        op=ALU.add).then_inc(ev3, 1)

    return nc


def _prep_weights(W_ih, W_hh, b_ih, b_hh, head_w, head_b):
    # gate order (i,f,g,o) -> (i,f,o,g)
    perm = np.concatenate([np.arange(0, 2 * H), np.arange(3 * H, 4 * H),
                           np.arange(2 * H, 3 * H)])
    wi = W_ih[perm].astype(np.float64).copy()
    wh = W_hh[perm].astype(np.float64).copy()
    bb = (b_ih + b_hh)[perm].astype(np.float64).copy()
    # tanh fold: g rows x2 everywhere; h stored as h/2: W_hh x2, head_w x2
    wi[3 * H:] *= 2.0
    bb[3 * H:] *= 2.0
    wh *= 2.0
    wh[3 * H:] *= 2.0
    hwn = 2.0 * head_w.astype(np.float64)

    wihT = wi.T.astype(BF16)                       # [D, G4]
    whhT_f = wh.T                                  # [H, G4]
    whh_tiles = np.zeros((128, 16 * 128), np.float64)
    for k in range(2):
        for m in range(8):
            whh_tiles[:, (k * 8 + m) * 128:(k * 8 + m + 1) * 128] = \
                whhT_f[k * 128:(k + 1) * 128, m * 128:(m + 1) * 128]
    hwT = hwn.reshape(NVV, H).T                    # [H, NVV]
    hw_tiles = np.concatenate([hwT[:128], hwT[128:]], axis=1)  # [128, 2*NVV]
    biasm = bb.reshape(8, 128).T.astype(np.float32).copy()     # [128, 8]
    headb = np.broadcast_to(head_b.reshape(NVV)[None, :],
                            (128, NVV)).astype(np.float32).copy()
    ident = np.eye(128, dtype=BF16)
    return (np.ascontiguousarray(wihT),
            np.ascontiguousarray(whh_tiles.astype(BF16)),
            np.ascontiguousarray(hw_tiles.astype(BF16)),
            biasm, headb, ident)


def kernel(x, W_ih, W_hh, b_ih, b_hh, head_w, head_b):
    x = np.asarray(x)
    wihT, whh_tiles, hw_tiles, biasm, headb, ident = _prep_weights(
        np.asarray(W_ih), np.asarray(W_hh), np.asarray(b_ih),
        np.asarray(b_hh), np.asarray(head_w), np.asarray(head_b))

    if "nc" not in _cache:
        _cache["nc"] = build_nc(T)
    nc = _cache["nc"]

    consts = np.broadcast_to(np.array([0.5, 2.0], np.float32)[None, :],
                             (128, 2)).copy()
    in_maps = []
    for c in range(NC):
        xs = x[c * BC:(c + 1) * BC]                    # [BC, T, D]
        xTc = np.ascontiguousarray(
            xs.transpose(2, 1, 0).reshape(128, BC * T)).astype(BF16)
        in_maps.append({
            "xT": xTc, "wihT": wihT, "whhT": whh_tiles, "headwT": hw_tiles,
            "biasm": biasm, "headb": headb, "ident": ident,
            "consts": consts, "zeros16": np.zeros((128, 16), np.float32),
        })

    res = run_bass_kernel_spmd(nc, in_maps, core_ids=list(range(NC)))
    outs = [r["logits"] for r in res.results]          # [NV, BC, T, V] each
    full = np.concatenate(outs, axis=1)                # [NV, B, T, V]
    return (full[0], full[1], full[2])

